# revision 9
# baseline (speedup 1.0000x reference)
"""AFT-Full attention kernel for 8 TRN2 NeuronCores.

Data-parallel over batch B=8 (one batch element per core). Per core:
  Q = x_q @ wq + wq_b          [2048, 256]
  K = x_kv @ wk + wk_b         [2048, 256]
  V = x_kv @ wv + wv_b         [2048, 256]
  num = exp(bias) @ (exp(K)*V) [2048, 256]
  den = exp(bias) @ exp(K)     [2048, 256]
  Yt  = sigmoid(Q) * num / den
  out = Yt @ f2_w + f2_b       [2048, 256]

On-chip layout: Q/num/den are produced transposed ([h, s], h on partitions)
so that Yt^T is directly the lhsT for the final f2 matmul. K/V are produced
natural ([t, h]) so they serve as lhsT for the num/den matmuls. All inputs
are transposed on the TensorEngine (identity-matmul transpose) after a
f32->bf16 cast performed inside the SWDGE DMA. Transposes and matmuls are
interleaved per k-tile so the PE never waits on the PSUM->SBUF copies.
"""

import numpy as np
from contextlib import ExitStack

import concourse.bass as bass
import concourse.tile as tile
from concourse import bacc, mybir
from concourse.bass_utils import run_bass_kernel_spmd
from concourse.masks import make_identity

F32 = mybir.dt.float32
BF16 = mybir.dt.bfloat16

S = 2048   # n_q
T = 2048   # n_kv
D = 1024   # d_q == d_kv
H = 256    # hidden
G = 256    # output dim
P = 128    # partitions
SCH = 512  # s-chunk for phase C (one PSUM bank of fp32)
NCH = S // SCH       # 4 chunks
NT = T // P          # 16 t row-blocks
NG = NT // 2         # 8 groups of 2 t-blocks
ND = D // P          # 8 d tiles
NHB = H // P         # 2 h blocks

AFT = mybir.ActivationFunctionType


def _build(use_wq_b, use_wk_b, use_wv_b, use_f2_b):
    """Build the per-core Bass graph. Returns the compiled Bacc."""
    nc = bacc.Bacc(
        "TRN2",
        target_bir_lowering=False,
        debug=False,
        enable_asserts=False,
        num_devices=8,
    )

    x_q = nc.declare_dram_parameter("x_q", [S, D], F32, isOutput=False)
    x_kv = nc.declare_dram_parameter("x_kv", [T, D], F32, isOutput=False)
    bias = nc.declare_dram_parameter("bias", [S, T], F32, isOutput=False)
    wq_w = nc.declare_dram_parameter("wq_w", [D, H], F32, isOutput=False)
    wk_w = nc.declare_dram_parameter("wk_w", [D, H], F32, isOutput=False)
    wv_w = nc.declare_dram_parameter("wv_w", [D, H], F32, isOutput=False)
    f2_w = nc.declare_dram_parameter("f2_w", [H, G], F32, isOutput=False)
    wq_b = nc.declare_dram_parameter("wq_b", [1, H], F32, isOutput=False) if use_wq_b else None
    wk_b = nc.declare_dram_parameter("wk_b", [1, H], F32, isOutput=False) if use_wk_b else None
    wv_b = nc.declare_dram_parameter("wv_b", [1, H], F32, isOutput=False) if use_wv_b else None
    f2_b = nc.declare_dram_parameter("f2_b", [1, G], F32, isOutput=False) if use_f2_b else None
    out = nc.declare_dram_parameter("out", [S, G], F32, isOutput=True)

    with tile.TileContext(nc) as tc, ExitStack() as ctx:
        consts = ctx.enter_context(tc.tile_pool(name="consts", bufs=1))

        ident = consts.tile([P, P], BF16)
        make_identity(nc, ident[:])

        # Staging pools.
        xq_nat = ctx.enter_context(tc.tile_pool(name="xq_nat", bufs=6))
        xkv_nat = ctx.enter_context(tc.tile_pool(name="xkv_nat", bufs=4))
        xT_sb = ctx.enter_context(tc.tile_pool(name="xT_sb", bufs=10))
        bias_nat = ctx.enter_context(tc.tile_pool(name="bias_nat", bufs=8))
        expw_sb = ctx.enter_context(tc.tile_pool(name="expw_sb", bufs=4))
        epi = ctx.enter_context(tc.tile_pool(name="epi", bufs=4))
        out_sb_pool = ctx.enter_context(tc.tile_pool(name="out_sb", bufs=2))

        # First x_q chunk loads go first so the PE can start ASAP; weights
        # follow (they are only needed once the first transposes are done).
        first_xq = []
        for sb in range(SCH // P):
            t_ = xq_nat.tile([P, D], BF16, tag="xq_nat", name="xq_nat")
            nc.gpsimd.dma_start(t_[:], x_q[sb * P : (sb + 1) * P, :])
            first_xq.append(t_)

        # Weights: [128, ND, H] bf16, slice [:, d, :] is the d-th k-tile.
        w_sb = {}
        for name, w in (("wq", wq_w), ("wk", wk_w), ("wv", wv_w)):
            t_ = consts.tile([P, ND, H], BF16, tag=f"w_{name}")
            nc.gpsimd.dma_start(t_[:], w[:].rearrange("(dt p) h -> p dt h", p=P))
            w_sb[name] = t_
        f2_sb = consts.tile([P, NHB, G], BF16, tag="w_f2")
        nc.gpsimd.dma_start(f2_sb[:], f2_w[:].rearrange("(ht p) g -> p ht g", p=P))

        bias_vecs = {}
        ones_row = None
        if any(b is not None for b in (wq_b, wk_b, wv_b, f2_b)):
            ones_row = consts.tile([1, SCH], BF16)
            nc.gpsimd.memset(ones_row[:], 1.0)
            for name, b in (("wq", wq_b), ("wk", wk_b), ("wv", wv_b), ("f2", f2_b)):
                if b is not None:
                    bt = consts.tile([1, H], BF16, tag=f"b_{name}")
                    nc.gpsimd.dma_start(bt[:], b[:])
                    bias_vecs[name] = bt

        # Long-lived activations.
        sigq_pool = ctx.enter_context(tc.tile_pool(name="sigq", bufs=NHB))
        sigq = [sigq_pool.tile([P, S], BF16, tag="sigq", name="sigq") for _ in range(NHB)]
        ek_pool = ctx.enter_context(tc.tile_pool(name="expk", bufs=NG))
        ekv_pool = ctx.enter_context(tc.tile_pool(name="ekv", bufs=NG))
        expk = [ek_pool.tile([P, 2 * H], BF16, tag="expk", name="expk") for _ in range(NG)]
        ekv = [ekv_pool.tile([P, 2 * H], BF16, tag="ekv", name="ekv") for _ in range(NG)]

        # ---------------- Phases B and A ----------------
        with (
            tc.tile_pool(name="psum_trab", bufs=2, space="PSUM") as psum_trab,
            tc.tile_pool(name="psum_acc", bufs=6, space="PSUM") as psum_acc,
        ):
            # Phase B: Q^T + sigmoid, one 512-col chunk at a time.
            for c in range(NCH):
                if c == 0:
                    nat = first_xq
                else:
                    nat = []
                    for sb in range(SCH // P):
                        t_ = xq_nat.tile([P, D], BF16, tag="xq_nat", name="xq_nat")
                        r0 = c * SCH + sb * P
                        nc.gpsimd.dma_start(t_[:], x_q[r0 : r0 + P, :])
                        nat.append(t_)
                ps_q = [
                    psum_acc.tile([P, SCH], F32, tag="acc", name="ps_q")
                    for _ in range(NHB)
                ]
                for d in range(ND):
                    ps = psum_trab.tile([P, SCH], BF16, tag="tr", name="ps_tr")
                    for sb in range(SCH // P):
                        nc.tensor.transpose(
                            ps[:, sb * P : (sb + 1) * P],
                            nat[sb][:, d * P : (d + 1) * P],
                            ident[:],
                        )
                    xqT = xT_sb.tile([P, SCH], BF16, tag="xqT", name="xqT")
                    nc.vector.tensor_copy(xqT[:], ps[:])
                    for hb in range(NHB):
                        nc.tensor.matmul(
                            ps_q[hb][:],
                            w_sb["wq"][:, d, hb * P : (hb + 1) * P],
                            xqT[:],
                            start=(d == 0),
                            stop=(d == ND - 1 and "wq" not in bias_vecs),
                        )
                for hb in range(NHB):
                    if "wq" in bias_vecs:
                        nc.tensor.matmul(
                            ps_q[hb][:],
                            bias_vecs["wq"][:, hb * P : (hb + 1) * P],
                            ones_row[:],
                            start=False,
                            stop=True,
                        )
                    nc.scalar.activation(
                        sigq[hb][:, c * SCH : (c + 1) * SCH], ps_q[hb][:], AFT.Sigmoid
                    )

            # Phase A: K/V, exp(K), exp(K)*V, two t-blocks per group.
            for g in range(NG):
                nat = []
                for j in range(2):
                    t_ = xkv_nat.tile([P, D], BF16, tag="xkv_nat", name="xkv_nat")
                    r0 = (g * 2 + j) * P
                    nc.gpsimd.dma_start(t_[:], x_kv[r0 : r0 + P, :])
                    nat.append(t_)
                ps_k = [psum_acc.tile([P, H], F32, tag="acc", name="ps_k") for _ in range(2)]
                ps_v = [psum_acc.tile([P, H], F32, tag="acc", name="ps_v") for _ in range(2)]
                for d in range(ND):
                    ps = psum_trab.tile([P, SCH], BF16, tag="tr", name="ps_tr")
                    for j in range(2):
                        nc.tensor.transpose(
                            ps[:, j * P : (j + 1) * P],
                            nat[j][:, d * P : (d + 1) * P],
                            ident[:],
                        )
                    xkvT = xT_sb.tile([P, 2 * P], BF16, tag="xkvT", name="xkvT")
                    nc.vector.tensor_copy(xkvT[:], ps[:, 0 : 2 * P])
                    for j in range(2):
                        for ps_o, wname in ((ps_k, "wk"), (ps_v, "wv")):
                            nc.tensor.matmul(
                                ps_o[j][:],
                                xkvT[:, j * P : (j + 1) * P],
                                w_sb[wname][:, d, :],
                                start=(d == 0),
                                stop=(d == ND - 1 and wname not in bias_vecs),
                            )
                if bias_vecs:
                    for j in range(2):
                        for ps_o, wname in ((ps_k, "wk"), (ps_v, "wv")):
                            if wname in bias_vecs:
                                nc.tensor.matmul(
                                    ps_o[j][:],
                                    ones_row[:, 0:P],
                                    bias_vecs[wname][:],
                                    start=False,
                                    stop=True,
                                )
                for j in range(2):
                    nc.scalar.activation(
                        expk[g][:, j * H : (j + 1) * H], ps_k[j][:], AFT.Exp
                    )
                    nc.vector.tensor_mul(
                        ekv[g][:, j * H : (j + 1) * H],
                        expk[g][:, j * H : (j + 1) * H],
                        ps_v[j][:],
                    )

        # ---------------- Phase C: exp(bias)^T, num/den, epilogue ----------------
        with (
            tc.tile_pool(name="psum_trc", bufs=2, space="PSUM") as psum_trc,
            tc.tile_pool(name="psum_nd", bufs=4, space="PSUM") as psum_nd,
            tc.tile_pool(name="psum_f2", bufs=2, space="PSUM") as psum_f2,
        ):
            for c in range(NCH):
                nat = []
                for sb in range(SCH // P):
                    t_ = bias_nat.tile([P, T], BF16, tag="bias_nat", name="bias_nat")
                    r0 = c * SCH + sb * P
                    nc.gpsimd.dma_start(t_[:], bias[r0 : r0 + P, :])
                    nat.append(t_)
                ps_num = [psum_nd.tile([P, SCH], F32, tag="nd", name="ps_num") for _ in range(NHB)]
                ps_den = [psum_nd.tile([P, SCH], F32, tag="nd", name="ps_den") for _ in range(NHB)]
                expw = []

                def emit_tr(g, nat=nat, expw=expw):
                    ps = psum_trc.tile([P, 2 * SCH], BF16, tag="trc", name="ps_trc")
                    for j in range(2):
                        tb = g * 2 + j
                        for sb in range(SCH // P):
                            nc.tensor.transpose(
                                ps[:, j * SCH + sb * P : j * SCH + (sb + 1) * P],
                                nat[sb][:, tb * P : (tb + 1) * P],
                                ident[:],
                            )
                    sbuf = expw_sb.tile([P, 2 * SCH], BF16, tag="expw", name="expw")
                    nc.scalar.activation(sbuf[:], ps[:], AFT.Exp)
                    expw.append(sbuf)

                def emit_mm(g, c=c, ps_num=ps_num, ps_den=ps_den, expw=expw):
                    for j in range(2):
                        tb = g * 2 + j
                        rhs = expw[g][:, j * SCH : (j + 1) * SCH]
                        for hb in range(NHB):
                            lo = j * H + hb * P
                            nc.tensor.matmul(
                                ps_num[hb][:],
                                ekv[g][:, lo : lo + P],
                                rhs,
                                start=(tb == 0),
                                stop=(tb == NT - 1),
                            )
                            nc.tensor.matmul(
                                ps_den[hb][:],
                                expk[g][:, lo : lo + P],
                                rhs,
                                start=(tb == 0),
                                stop=(tb == NT - 1),
                            )

                # Stagger transpose/exp one group ahead of the matmuls so the
                # PE always has matmul work while ACT computes the next exp.
                emit_tr(0)
                for g in range(1, NG):
                    emit_tr(g)
                    emit_mm(g - 1)
                emit_mm(NG - 1)

                ytT = []
                for hb in range(NHB):
                    rec = epi.tile([P, SCH], F32, tag="rec", name="rec")
                    nc.vector.reciprocal_approx_fast(rec[:], ps_den[hb][:])
                    tmp = epi.tile([P, SCH], BF16, tag="tmp", name="tmp")
                    nc.vector.tensor_mul(tmp[:], rec[:], ps_num[hb][:])
                    yt = epi.tile([P, SCH], BF16, tag="yt", name="yt")
                    nc.vector.tensor_mul(
                        yt[:], tmp[:], sigq[hb][:, c * SCH : (c + 1) * SCH]
                    )
                    ytT.append(yt)

                out_sb = out_sb_pool.tile([P, SCH // P, G], F32, tag="out_sb", name="out_sb")
                for sb in range(SCH // P):
                    ps_f = psum_f2.tile([P, G], F32, tag="f2", name="ps_f")
                    for hb in range(NHB):
                        nc.tensor.matmul(
                            ps_f[:],
                            ytT[hb][:, sb * P : (sb + 1) * P],
                            f2_sb[:, hb, :],
                            start=(hb == 0),
                            stop=(hb == NHB - 1 and "f2" not in bias_vecs),
                        )
                    if "f2" in bias_vecs:
                        nc.tensor.matmul(
                            ps_f[:],
                            ones_row[:, 0:P],
                            bias_vecs["f2"][:],
                            start=False,
                            stop=True,
                        )
                    nc.vector.tensor_copy(out_sb[:, sb, :], ps_f[:])
                nc.sync.dma_start(
                    out[:].rearrange("(cc sb p) g -> cc p sb g", p=P, sb=SCH // P)[c],
                    out_sb[:],
                )

    nc.compile()
    return nc


_CACHE = {}


def _get_nc(use_wq_b, use_wk_b, use_wv_b, use_f2_b):
    key = (use_wq_b, use_wk_b, use_wv_b, use_f2_b)
    if key not in _CACHE:
        _CACHE[key] = _build(*key)
    return _CACHE[key]


def kernel(x_q, x_kv, bias, wq_w, wq_b, wk_w, wk_b, wv_w, wv_b, f2_w, f2_b,
           _trace=False, _trace_kwargs=None):
    x_q = np.ascontiguousarray(np.asarray(x_q, dtype=np.float32))
    x_kv = np.ascontiguousarray(np.asarray(x_kv, dtype=np.float32))
    bias = np.ascontiguousarray(np.asarray(bias, dtype=np.float32))
    wq_w = np.ascontiguousarray(np.asarray(wq_w, dtype=np.float32))
    wk_w = np.ascontiguousarray(np.asarray(wk_w, dtype=np.float32))
    wv_w = np.ascontiguousarray(np.asarray(wv_w, dtype=np.float32))
    f2_w = np.ascontiguousarray(np.asarray(f2_w, dtype=np.float32))
    wq_b = np.asarray(wq_b, dtype=np.float32)
    wk_b = np.asarray(wk_b, dtype=np.float32)
    wv_b = np.asarray(wv_b, dtype=np.float32)
    f2_b = np.asarray(f2_b, dtype=np.float32)

    use_b = tuple(bool(np.any(b)) for b in (wq_b, wk_b, wv_b, f2_b))
    nc = _get_nc(*use_b)

    n_cores = 8
    in_maps = []
    for i in range(n_cores):
        m = {
            "x_q": x_q[i],
            "x_kv": x_kv[i],
            "bias": bias[i],
            "wq_w": wq_w,
            "wk_w": wk_w,
            "wv_w": wv_w,
            "f2_w": f2_w,
        }
        if use_b[0]:
            m["wq_b"] = wq_b.reshape(1, H)
        if use_b[1]:
            m["wk_b"] = wk_b.reshape(1, H)
        if use_b[2]:
            m["wv_b"] = wv_b.reshape(1, H)
        if use_b[3]:
            m["f2_b"] = f2_b.reshape(1, G)
        in_maps.append(m)

    res = run_bass_kernel_spmd(
        nc, in_maps, list(range(n_cores)), trace=_trace, **(_trace_kwargs or {})
    )
    out = np.stack([np.asarray(res.results[i]["out"]) for i in range(n_cores)], axis=0)
    if _trace:
        return out, res
    return out


# revision 10
# speedup vs baseline: 1.1220x; 1.1220x over previous
"""AFT-Full attention kernel for 8 TRN2 NeuronCores.

Data-parallel over batch B=8 (one batch element per core). Per core:
  Q = x_q @ wq + wq_b          [2048, 256]
  K = x_kv @ wk + wk_b         [2048, 256]
  V = x_kv @ wv + wv_b         [2048, 256]
  num = exp(bias) @ (exp(K)*V) [2048, 256]
  den = exp(bias) @ exp(K)     [2048, 256]
  Yt  = sigmoid(Q) * num / den
  out = Yt @ f2_w + f2_b       [2048, 256]

On-chip layout: Q/num/den are produced transposed ([h, s], h on partitions)
so that Yt^T is directly the lhsT for the final f2 matmul. K/V are produced
natural ([t, h]) so they serve as lhsT for the num/den matmuls. All inputs
are transposed on the TensorEngine (identity-matmul transpose) after a
f32->bf16 cast performed inside the SWDGE DMA. Transposes and matmuls run
in large bursts (interleaving them costs PE mode switches + HAM cooling).
"""

import numpy as np
from contextlib import ExitStack

import concourse.bass as bass
import concourse.tile as tile
from concourse import bacc, mybir
from concourse.bass_utils import run_bass_kernel_spmd
from concourse.masks import make_identity

F32 = mybir.dt.float32
BF16 = mybir.dt.bfloat16

S = 2048   # n_q
T = 2048   # n_kv
D = 1024   # d_q == d_kv
H = 256    # hidden
G = 256    # output dim
P = 128    # partitions
SCH = 512  # s-chunk for phase C (one PSUM bank of fp32)
NSB = SCH // P       # 4 row-blocks per chunk
NCH = S // SCH       # 4 chunks
NT = T // P          # 16 t row-blocks
NG = NT // 2         # 8 groups of 2 t-blocks
ND = D // P          # 8 d tiles
NHB = H // P         # 2 h blocks

AFT = mybir.ActivationFunctionType


def _build(use_wq_b, use_wk_b, use_wv_b, use_f2_b):
    """Build the per-core Bass graph. Returns the compiled Bacc."""
    nc = bacc.Bacc(
        "TRN2",
        target_bir_lowering=False,
        debug=False,
        enable_asserts=False,
        num_devices=8,
    )

    x_q = nc.declare_dram_parameter("x_q", [S, D], F32, isOutput=False)
    x_kv = nc.declare_dram_parameter("x_kv", [T, D], F32, isOutput=False)
    bias = nc.declare_dram_parameter("bias", [S, T], F32, isOutput=False)
    wq_w = nc.declare_dram_parameter("wq_w", [D, H], F32, isOutput=False)
    wk_w = nc.declare_dram_parameter("wk_w", [D, H], F32, isOutput=False)
    wv_w = nc.declare_dram_parameter("wv_w", [D, H], F32, isOutput=False)
    f2_w = nc.declare_dram_parameter("f2_w", [H, G], F32, isOutput=False)
    wq_b = nc.declare_dram_parameter("wq_b", [1, H], F32, isOutput=False) if use_wq_b else None
    wk_b = nc.declare_dram_parameter("wk_b", [1, H], F32, isOutput=False) if use_wk_b else None
    wv_b = nc.declare_dram_parameter("wv_b", [1, H], F32, isOutput=False) if use_wv_b else None
    f2_b = nc.declare_dram_parameter("f2_b", [1, G], F32, isOutput=False) if use_f2_b else None
    out = nc.declare_dram_parameter("out", [S, G], F32, isOutput=True)

    # DRAM views with the chunk/block structure exposed.
    xq_v = x_q[:].rearrange("(c sb p) d -> c p sb d", p=P, sb=NSB)     # [NCH,P,NSB,D]
    xkv_v = x_kv[:].rearrange("(g j p) d -> g p j d", p=P, j=2)        # [NG,P,2,D]
    bias_v = bias[:].rearrange("(c sb p) t -> c p sb t", p=P, sb=NSB)  # [NCH,P,NSB,T]
    out_v = out[:].rearrange("(c sb p) g -> c p sb g", p=P, sb=NSB)

    with tile.TileContext(nc) as tc, ExitStack() as ctx:
        consts = ctx.enter_context(tc.tile_pool(name="consts", bufs=1))

        ident = consts.tile([P, P], BF16)
        make_identity(nc, ident[:])

        # Staging pools (one tile per chunk/group, cast to bf16 in the DMA).
        xq_nat = ctx.enter_context(tc.tile_pool(name="xq_nat", bufs=3))
        xkv_nat = ctx.enter_context(tc.tile_pool(name="xkv_nat", bufs=3))
        xT_sb = ctx.enter_context(tc.tile_pool(name="xT_sb", bufs=10))
        bias_nat = ctx.enter_context(tc.tile_pool(name="bias_nat", bufs=3))
        expw_sb = ctx.enter_context(tc.tile_pool(name="expw_sb", bufs=4))
        epi = ctx.enter_context(tc.tile_pool(name="epi", bufs=4))
        out_sb_pool = ctx.enter_context(tc.tile_pool(name="out_sb", bufs=2))

        def load_xq(c):
            t_ = xq_nat.tile([P, NSB, D], BF16, tag="xq_nat", name="xq_nat")
            nc.gpsimd.dma_start(t_[:], xq_v[c])
            return t_

        # First x_q chunk goes first so the PE can start ASAP; weights
        # follow (they are only needed once the first transposes are done).
        first_xq = load_xq(0)

        # Weights: [128, ND, H] bf16, slice [:, d, :] is the d-th k-tile.
        w_sb = {}
        for name, w in (("wq", wq_w), ("wk", wk_w), ("wv", wv_w)):
            t_ = consts.tile([P, ND, H], BF16, tag=f"w_{name}")
            nc.gpsimd.dma_start(t_[:], w[:].rearrange("(dt p) h -> p dt h", p=P))
            w_sb[name] = t_
        f2_sb = consts.tile([P, NHB, G], BF16, tag="w_f2")
        nc.gpsimd.dma_start(f2_sb[:], f2_w[:].rearrange("(ht p) g -> p ht g", p=P))

        bias_vecs = {}
        ones_row = None
        if any(b is not None for b in (wq_b, wk_b, wv_b, f2_b)):
            ones_row = consts.tile([1, SCH], BF16)
            nc.gpsimd.memset(ones_row[:], 1.0)
            for name, b in (("wq", wq_b), ("wk", wk_b), ("wv", wv_b), ("f2", f2_b)):
                if b is not None:
                    bt = consts.tile([1, H], BF16, tag=f"b_{name}")
                    nc.gpsimd.dma_start(bt[:], b[:])
                    bias_vecs[name] = bt

        # Long-lived activations.
        sigq_pool = ctx.enter_context(tc.tile_pool(name="sigq", bufs=NHB))
        sigq = [sigq_pool.tile([P, S], BF16, tag="sigq", name="sigq") for _ in range(NHB)]
        ek_pool = ctx.enter_context(tc.tile_pool(name="expk", bufs=NG))
        ekv_pool = ctx.enter_context(tc.tile_pool(name="ekv", bufs=NG))
        expk = [ek_pool.tile([P, 2 * H], BF16, tag="expk", name="expk") for _ in range(NG)]
        ekv = [ekv_pool.tile([P, 2 * H], BF16, tag="ekv", name="ekv") for _ in range(NG)]

        # ---------------- Phases B and A ----------------
        with (
            tc.tile_pool(name="psum_trab", bufs=3, space="PSUM") as psum_trab,
            tc.tile_pool(name="psum_acc", bufs=5, space="PSUM") as psum_acc,
        ):
            # Phase B: Q^T + sigmoid, one 512-col chunk at a time.
            for c in range(NCH):
                nat = first_xq if c == 0 else load_xq(c)
                # Transpose burst: x_q[c]^T into SBUF, one [P, SCH] tile per d.
                xqT = []
                for d in range(ND):
                    ps = psum_trab.tile([P, SCH], BF16, tag="tr", name="ps_tr")
                    for sb in range(NSB):
                        nc.tensor.transpose(
                            ps[:, sb * P : (sb + 1) * P],
                            nat[:, sb, d * P : (d + 1) * P],
                            ident[:],
                        )
                    sbuf = xT_sb.tile([P, SCH], BF16, tag="xqT", name="xqT")
                    nc.vector.tensor_copy(sbuf[:], ps[:])
                    xqT.append(sbuf)
                # Matmul burst.
                ps_q = [
                    psum_acc.tile([P, SCH], F32, tag="acc", name="ps_q")
                    for _ in range(NHB)
                ]
                for d in range(ND):
                    for hb in range(NHB):
                        nc.tensor.matmul(
                            ps_q[hb][:],
                            w_sb["wq"][:, d, hb * P : (hb + 1) * P],
                            xqT[d][:],
                            start=(d == 0),
                            stop=(d == ND - 1 and "wq" not in bias_vecs),
                        )
                for hb in range(NHB):
                    if "wq" in bias_vecs:
                        nc.tensor.matmul(
                            ps_q[hb][:],
                            bias_vecs["wq"][:, hb * P : (hb + 1) * P],
                            ones_row[:],
                            start=False,
                            stop=True,
                        )
                    nc.scalar.activation(
                        sigq[hb][:, c * SCH : (c + 1) * SCH], ps_q[hb][:], AFT.Sigmoid
                    )

            # Phase A: K/V, exp(K), exp(K)*V, two t-blocks per group.
            for g in range(NG):
                nat = xkv_nat.tile([P, 2, D], BF16, tag="xkv_nat", name="xkv_nat")
                nc.gpsimd.dma_start(nat[:], xkv_v[g])
                xkvT = []
                for d in range(ND):
                    ps = psum_trab.tile([P, SCH], BF16, tag="tr", name="ps_tr")
                    for j in range(2):
                        nc.tensor.transpose(
                            ps[:, j * P : (j + 1) * P],
                            nat[:, j, d * P : (d + 1) * P],
                            ident[:],
                        )
                    sbuf = xT_sb.tile([P, 2 * P], BF16, tag="xkvT", name="xkvT")
                    nc.vector.tensor_copy(sbuf[:], ps[:, 0 : 2 * P])
                    xkvT.append(sbuf)
                ps_k = [psum_acc.tile([P, H], F32, tag="acc", name="ps_k") for _ in range(2)]
                ps_v = [psum_acc.tile([P, H], F32, tag="acc", name="ps_v") for _ in range(2)]
                for d in range(ND):
                    for j in range(2):
                        for ps_o, wname in ((ps_k, "wk"), (ps_v, "wv")):
                            nc.tensor.matmul(
                                ps_o[j][:],
                                xkvT[d][:, j * P : (j + 1) * P],
                                w_sb[wname][:, d, :],
                                start=(d == 0),
                                stop=(d == ND - 1 and wname not in bias_vecs),
                            )
                if bias_vecs:
                    for j in range(2):
                        for ps_o, wname in ((ps_k, "wk"), (ps_v, "wv")):
                            if wname in bias_vecs:
                                nc.tensor.matmul(
                                    ps_o[j][:],
                                    ones_row[:, 0:P],
                                    bias_vecs[wname][:],
                                    start=False,
                                    stop=True,
                                )
                for j in range(2):
                    nc.scalar.activation(
                        expk[g][:, j * H : (j + 1) * H], ps_k[j][:], AFT.Exp
                    )
                    nc.vector.tensor_mul(
                        ekv[g][:, j * H : (j + 1) * H],
                        expk[g][:, j * H : (j + 1) * H],
                        ps_v[j][:],
                    )

        # ---------------- Phase C: exp(bias)^T, num/den, epilogue ----------------
        with (
            tc.tile_pool(name="psum_trc", bufs=3, space="PSUM") as psum_trc,
            tc.tile_pool(name="psum_nd", bufs=4, space="PSUM") as psum_nd,
            tc.tile_pool(name="psum_f2", bufs=1, space="PSUM") as psum_f2,
        ):
            for c in range(NCH):
                nat = bias_nat.tile([P, NSB, T], BF16, tag="bias_nat", name="bias_nat")
                nc.gpsimd.dma_start(nat[:], bias_v[c])
                # Transpose + exp burst: exp(bias[c])^T, two t-blocks per tile.
                expw = []
                for g in range(NG):
                    ps = psum_trc.tile([P, 2 * SCH], BF16, tag="trc", name="ps_trc")
                    for j in range(2):
                        tb = g * 2 + j
                        for sb in range(NSB):
                            nc.tensor.transpose(
                                ps[:, j * SCH + sb * P : j * SCH + (sb + 1) * P],
                                nat[:, sb, tb * P : (tb + 1) * P],
                                ident[:],
                            )
                    sbuf = expw_sb.tile([P, 2 * SCH], BF16, tag="expw", name="expw")
                    nc.scalar.activation(sbuf[:], ps[:], AFT.Exp)
                    expw.append(sbuf)
                # Matmul burst: num/den accumulation over all 16 t-blocks.
                ps_num = [psum_nd.tile([P, SCH], F32, tag="nd", name="ps_num") for _ in range(NHB)]
                ps_den = [psum_nd.tile([P, SCH], F32, tag="nd", name="ps_den") for _ in range(NHB)]
                for g in range(NG):
                    for j in range(2):
                        tb = g * 2 + j
                        rhs = expw[g][:, j * SCH : (j + 1) * SCH]
                        for hb in range(NHB):
                            lo = j * H + hb * P
                            nc.tensor.matmul(
                                ps_num[hb][:],
                                ekv[g][:, lo : lo + P],
                                rhs,
                                start=(tb == 0),
                                stop=(tb == NT - 1),
                            )
                            nc.tensor.matmul(
                                ps_den[hb][:],
                                expk[g][:, lo : lo + P],
                                rhs,
                                start=(tb == 0),
                                stop=(tb == NT - 1),
                            )
                # Epilogue: Yt^T = sigmoid(Q)^T * num^T / den^T.
                ytT = []
                for hb in range(NHB):
                    rec = epi.tile([P, SCH], F32, tag="rec", name="rec")
                    nc.vector.reciprocal_approx_fast(rec[:], ps_den[hb][:])
                    tmp = epi.tile([P, SCH], BF16, tag="tmp", name="tmp")
                    nc.vector.tensor_mul(tmp[:], rec[:], ps_num[hb][:])
                    yt = epi.tile([P, SCH], BF16, tag="yt", name="yt")
                    nc.vector.tensor_mul(
                        yt[:], tmp[:], sigq[hb][:, c * SCH : (c + 1) * SCH]
                    )
                    ytT.append(yt)
                # f2 projection + store.
                out_sb = out_sb_pool.tile([P, NSB, G], F32, tag="out_sb", name="out_sb")
                for sb in range(NSB):
                    ps_f = psum_f2.tile([P, G], F32, tag="f2", name="ps_f")
                    for hb in range(NHB):
                        nc.tensor.matmul(
                            ps_f[:],
                            ytT[hb][:, sb * P : (sb + 1) * P],
                            f2_sb[:, hb, :],
                            start=(hb == 0),
                            stop=(hb == NHB - 1 and "f2" not in bias_vecs),
                        )
                    if "f2" in bias_vecs:
                        nc.tensor.matmul(
                            ps_f[:],
                            ones_row[:, 0:P],
                            bias_vecs["f2"][:],
                            start=False,
                            stop=True,
                        )
                    nc.vector.tensor_copy(out_sb[:, sb, :], ps_f[:])
                nc.sync.dma_start(out_v[c], out_sb[:])

    nc.compile()
    return nc


_CACHE = {}


def _get_nc(use_wq_b, use_wk_b, use_wv_b, use_f2_b):
    key = (use_wq_b, use_wk_b, use_wv_b, use_f2_b)
    if key not in _CACHE:
        _CACHE[key] = _build(*key)
    return _CACHE[key]


def kernel(x_q, x_kv, bias, wq_w, wq_b, wk_w, wk_b, wv_w, wv_b, f2_w, f2_b,
           _trace=False, _trace_kwargs=None):
    x_q = np.ascontiguousarray(np.asarray(x_q, dtype=np.float32))
    x_kv = np.ascontiguousarray(np.asarray(x_kv, dtype=np.float32))
    bias = np.ascontiguousarray(np.asarray(bias, dtype=np.float32))
    wq_w = np.ascontiguousarray(np.asarray(wq_w, dtype=np.float32))
    wk_w = np.ascontiguousarray(np.asarray(wk_w, dtype=np.float32))
    wv_w = np.ascontiguousarray(np.asarray(wv_w, dtype=np.float32))
    f2_w = np.ascontiguousarray(np.asarray(f2_w, dtype=np.float32))
    wq_b = np.asarray(wq_b, dtype=np.float32)
    wk_b = np.asarray(wk_b, dtype=np.float32)
    wv_b = np.asarray(wv_b, dtype=np.float32)
    f2_b = np.asarray(f2_b, dtype=np.float32)

    use_b = tuple(bool(np.any(b)) for b in (wq_b, wk_b, wv_b, f2_b))
    nc = _get_nc(*use_b)

    n_cores = 8
    in_maps = []
    for i in range(n_cores):
        m = {
            "x_q": x_q[i],
            "x_kv": x_kv[i],
            "bias": bias[i],
            "wq_w": wq_w,
            "wk_w": wk_w,
            "wv_w": wv_w,
            "f2_w": f2_w,
        }
        if use_b[0]:
            m["wq_b"] = wq_b.reshape(1, H)
        if use_b[1]:
            m["wk_b"] = wk_b.reshape(1, H)
        if use_b[2]:
            m["wv_b"] = wv_b.reshape(1, H)
        if use_b[3]:
            m["f2_b"] = f2_b.reshape(1, G)
        in_maps.append(m)

    res = run_bass_kernel_spmd(
        nc, in_maps, list(range(n_cores)), trace=_trace, **(_trace_kwargs or {})
    )
    out = np.stack([np.asarray(res.results[i]["out"]) for i in range(n_cores)], axis=0)
    if _trace:
        return out, res
    return out
